# revision 2
# baseline (speedup 1.0000x reference)
"""Causal multi-head attention block on 8 Trainium2 NeuronCores.

Problem: x[4,2048,1024] -> QKV proj (16 heads, dh=64) -> causal softmax
attention -> out proj. Sharding: core = (batch, head-half): each core
computes QKV for 8 heads of one batch, flash-style attention for those
heads, and a partial O-projection over its 512 W_o input columns; the
host sums the two partials per batch (tensor-parallel unshard).

Device kernel (identical SPMD program, per-core data). Structure:
  - Q/K projection runs as fp8(e4m3) DoubleRow matmuls: weights are
    host-prescaled by 64 (to clear the e4m3 denormal range) and the
    1/64^2 is folded into the softmax exp scale. Contraction 1024 is
    4 DR matmuls of virtual-K 256 ([128 parts, 2 kd-chunks, .] APs on
    the natural [p, kd, t] tile layout -- no data shuffles). V and the
    O-projection stay bf16 (their error lands directly on the output).
  - scores are computed transposed, S.T[k_tile, q_span] = K.T_blk^T@Q.T,
    two k-tiles packed side by side in one 2-bank PSUM tile so ScalarE
    exps them in a single ACTIVATE; two heads (even/odd parity) run as
    concurrent row-tiled matmuls (auto tile_position from partition
    base 0/64). Diagonal blocks are masked after exp with a 0/1
    triangle multiply.
  - O.T[c, q] accumulates with V' stationary: V' has 64 V columns and
    64 ones-columns (parity-dependent order) so the matmul broadcasts
    the softmax denominator for free; normalization is one
    reciprocal_approx_fast + one cross-partition-base multiply.
  - Loop order is J-outer (q-chunk), head-pair-inner, so the partial
    O-projection for chunk J unlocks right after all head-pairs finish
    J and can be paced into the later, ScalarE-bound attention chunks
    as PE filler. QKV-projection units are likewise paced just before
    their consumers; input DMA is chunked so the first units' data
    lands first, with a short junk-matmul burst keeping the PE HAM
    clock-gate warm while the first chunks stream.
"""

import numpy as np
import ml_dtypes

BF16 = ml_dtypes.bfloat16
F8E4 = ml_dtypes.float8_e4m3

B, T, D = 4, 2048, 1024
NH, DH = 16, 64
HPC = 8            # heads per core
OC = HPC * DH      # 512: per-core head columns
NT = T // 128      # 16 q/k tiles of 128
ND = D // 128      # 8 d-tiles
N_CORES = 8
WSCALE = 64.0      # host pre-scale on W_qk/b_qk (e4m3 denormal dodge)

_cache = {}


def _build():
    import concourse.mybir as mybir
    import concourse.tile as tile
    from concourse import bacc

    f32 = mybir.dt.float32
    bf16 = mybir.dt.bfloat16
    fp8 = mybir.dt.float8e4
    Exp = mybir.ActivationFunctionType.Exp
    DR = mybir.MatmulPerfMode.DoubleRow

    nc = bacc.Bacc("TRN2", target_bir_lowering=False, debug=False,
                   num_devices=N_CORES)

    xT = nc.declare_dram_parameter("xT", [D, T], bf16, isOutput=False)
    x8 = nc.declare_dram_parameter("x8", [D, T], fp8, isOutput=False)
    w8 = nc.declare_dram_parameter("w8", [D, 2 * OC], fp8, isOutput=False)
    wv = nc.declare_dram_parameter("wvT", [D, OC], bf16, isOutput=False)
    wo = nc.declare_dram_parameter("woT", [OC, D], bf16, isOutput=False)
    bqk = nc.declare_dram_parameter("bqk", [128, 2 * OC // 128], f32, isOutput=False)
    bv = nc.declare_dram_parameter("bv", [1, OC], f32, isOutput=False)
    bo = nc.declare_dram_parameter("bo", [1, D], f32, isOutput=False)
    tri = nc.declare_dram_parameter("tri", [128, 128], bf16, isOutput=False)
    out = nc.declare_dram_parameter("out", [T, D], f32, isOutput=True)

    with tile.TileContext(nc) as tc:
        with (
            tc.tile_pool(name="persist", bufs=1) as persist,
            tc.tile_pool(name="pt", bufs=8) as ptp,
            tc.tile_pool(name="dn", bufs=6) as dnp,
            tc.tile_pool(name="ostage", bufs=4) as ostage,
            tc.tile_pool(name="psS", bufs=3, space="PSUM") as psS,
            tc.tile_pool(name="psO", bufs=2, space="PSUM") as psO,
        ):
            # ---- persistent SBUF tensors ----
            XT = persist.tile([128, ND, T], bf16)          # x.T d-tiles (V path)
            X8 = persist.tile([128, ND, T], fp8)           # x.T e4m3 (QK path)
            W8 = persist.tile([128, ND, 2 * OC], fp8)      # 64*W_qk.T e4m3
            WV = persist.tile([128, ND, OC], bf16)
            WO = persist.tile([128, OC // 128, D], bf16)
            BQK = persist.tile([128, 2 * OC // 128], f32)
            BV = persist.tile([128, OC], f32)
            BO = persist.tile([128, D], f32)
            TRI = persist.tile([128, 128], bf16)
            QKT = persist.tile([128, ND, T], bf16)         # [o, t] 64*(Q.T|K.T)
            # V' per head, 128 cols: even h: [V(64) | 1*64]; odd h:
            # [1*64 | V(64)]. O.T rows land on partitions (h%2)*64..+64 and
            # the other 64 rows all become the softmax denominator.
            VP = persist.tile([128, NT, HPC, 128], bf16)
            OT = persist.tile([128, OC // 128, T], bf16)   # attn out.T [c, t]

            # warm-up junk matmuls: keep the PE HAM clock-gate warm while
            # the first input DMA chunks stream in; results never read.
            JNK = persist.tile([128, 512], bf16)
            nc.vector.memset(JNK[:], 0.5)
            for g in range(3):
                jps = psS.tile([128, 512], f32, tag="s", name=f"jnk{g}")
                for m in range(8):
                    nc.tensor.matmul(
                        jps[:], lhsT=JNK[:, 0:128], rhs=JNK[:],
                        start=(m == 0), stop=(m == 7),
                    )

            # ---- input DMA, ordered by first use ----
            # sync queue: W8 (o-chunks), then X8 (t-chunks)
            w8r = w8.rearrange("(n p) o -> p n o", p=128)
            for oc in range(4):
                nc.sync.dma_start(out=W8[:, :, oc * 256:(oc + 1) * 256],
                                  in_=w8r[:, :, oc * 256:(oc + 1) * 256])
            x8r = x8.rearrange("(n p) t -> p n t", p=128)
            for tch in range(4):
                nc.sync.dma_start(out=X8[:, :, tch * 512:(tch + 1) * 512],
                                  in_=x8r[:, :, tch * 512:(tch + 1) * 512])
            # gpsimd queue: x.T bf16 in t-chunks (V path)
            xTr = xT.rearrange("(n p) t -> p n t", p=128)
            for tch in range(4):
                nc.gpsimd.dma_start(out=XT[:, :, tch * 512:(tch + 1) * 512],
                                    in_=xTr[:, :, tch * 512:(tch + 1) * 512])
            # scalar queue: V weights, biases, mask, then W_o
            nc.scalar.dma_start(out=WV[:], in_=wv.rearrange("(n p) o -> p n o", p=128))
            nc.scalar.dma_start(out=BQK[:], in_=bqk[:, :])
            nc.scalar.dma_start(out=TRI[:], in_=tri[:, :])
            nc.scalar.dma_start(out=BV[:], in_=bv[:, :].to_broadcast((128, OC)))
            nc.scalar.dma_start(out=BO[:], in_=bo[:, :].to_broadcast((128, D)))
            nc.scalar.dma_start(out=WO[:], in_=wo.rearrange("(n p) o -> p n o", p=128))
            nc.vector.memset(VP[:, :, 0:HPC:2, DH:128], 1.0)
            nc.vector.memset(VP[:, :, 1:HPC:2, 0:DH], 1.0)

            # ---- QKV projection units (PE filler) ----
            def emit_qk(ot, tch):
                # one [o, t] chunk: [128 o, 512 t] = 64*W_qk @ x.T + 64*b,
                # fp8 DoubleRow: 4 matmuls of virtual-K 256
                ps = psS.tile([128, 512], f32, tag="s",
                              name=f"qk{ot}_{tch}")
                for k in range(4):
                    nc.tensor.matmul(
                        ps[:],
                        lhsT=W8[:, 2 * k:2 * k + 2, ot * 128:(ot + 1) * 128],
                        rhs=X8[:, 2 * k:2 * k + 2, tch * 512:(tch + 1) * 512],
                        start=(k == 0), stop=(k == 3),
                        perf_mode=DR,
                    )
                nc.vector.tensor_scalar_add(
                    QKT[:, ot, tch * 512:(tch + 1) * 512], ps[:],
                    BQK[:, ot:ot + 1],
                )

            def emit_v(tt):
                # one [t, o] tile of V = x @ W_v.T + b, into parity layout
                ps = psS.tile([128, 512], f32, tag="s", name=f"v{tt}")
                for kd in range(ND):
                    nc.tensor.matmul(
                        ps[:],
                        lhsT=XT[:, kd, tt * 128:(tt + 1) * 128],
                        rhs=WV[:, kd, :],
                        start=(kd == 0), stop=(kd == ND - 1),
                    )
                nc.vector.tensor_tensor(
                    out=VP[:, tt, 0:HPC:2, 0:DH],
                    in0=ps[:].rearrange("p (a b) -> p a b", b=DH)[:, 0:HPC:2, :],
                    in1=BV[:].rearrange("p (a b) -> p a b", b=DH)[:, 0:HPC:2, :],
                    op=mybir.AluOpType.add,
                )
                nc.vector.tensor_tensor(
                    out=VP[:, tt, 1:HPC:2, DH:2 * DH],
                    in0=ps[:].rearrange("p (a b) -> p a b", b=DH)[:, 1:HPC:2, :],
                    in1=BV[:].rearrange("p (a b) -> p a b", b=DH)[:, 1:HPC:2, :],
                    op=mybir.AluOpType.add,
                )

            def emit_oproj(tq, oc2):
                # out[tq, oc2] = O @ WoT + 0.5 b_o (partial over this core's
                # 512 W_o input columns)
                ps = psS.tile([128, 512], f32, tag="s",
                              name=f"op{tq}_{oc2}")
                for ct in range(OC // 128):
                    nc.tensor.matmul(
                        ps[:],
                        lhsT=OT[:, ct, tq * 128:(tq + 1) * 128],
                        rhs=WO[:, ct, oc2 * 512:(oc2 + 1) * 512],
                        start=(ct == 0), stop=(ct == OC // 128 - 1),
                    )
                ob = ostage.tile([128, 512], f32, tag="ob")
                nc.vector.tensor_tensor(
                    out=ob[:], in0=ps[:],
                    in1=BO[:, oc2 * 512:(oc2 + 1) * 512],
                    op=mybir.AluOpType.add,
                )
                nc.sync.dma_start(
                    out=out[tq * 128:(tq + 1) * 128,
                            oc2 * 512:(oc2 + 1) * 512],
                    in_=ob[:],
                )

            # prologue (overlaps the input DMA): everything chunk J=0 needs
            emit_qk(0, 0)
            emit_qk(4, 0)
            prologue = [("v", 0), ("qk", 1, 0), ("qk", 5, 0), ("v", 1),
                        ("qk", 2, 0), ("qk", 6, 0), ("v", 2),
                        ("qk", 3, 0), ("qk", 7, 0), ("v", 3)]
            for u in prologue:
                if u[0] == "v":
                    emit_v(u[1])
                else:
                    emit_qk(u[1], u[2])

            # fill schedule keyed by global pop index. Pops happen once per
            # st_exp2 call: J=0 -> pops 0..7, J=1 -> 8..23, J=2 -> 24..47,
            # J=3 -> 48..79 (hp-major within J; (J,hp) starts at pop
            # J-base + hp*(2J+2)). Deadlines: qk(hp,J)/qk(4+hp,J) before
            # (J,hp) starts; v(4J..4J+3) before (J,hp0); oproj(J) anytime
            # after J completes -- deferred into the ScalarE-bound J=3.
            sched = {
                0: [("qk", 0, 1), ("qk", 4, 1)],
                1: [("qk", 1, 1), ("qk", 5, 1)],
                2: [("qk", 2, 1), ("qk", 6, 1)],
                3: [("qk", 3, 1), ("qk", 7, 1)],
                4: [("v", 4)], 5: [("v", 5)], 6: [("v", 6)], 7: [("v", 7)],
                # J=1 (pops 8..23): oproj(0) spread + J=2 prereqs
                8: [("op", 0, 0)], 9: [("op", 0, 1)],
                10: [("op", 1, 0)], 11: [("op", 1, 1)],
                12: [("op", 2, 0)], 13: [("op", 2, 1)],
                14: [("op", 3, 0)], 15: [("op", 3, 1)],
                16: [("qk", 0, 2)], 17: [("qk", 4, 2)],
                18: [("v", 8)], 19: [("v", 9)],
                20: [("v", 10)], 21: [("v", 11)],
                22: [("qk", 1, 2)], 23: [("qk", 5, 2)],
                # J=2 (pops 24..47): rest of qk(.,2), qk(.,3), v(12..15),
                # oproj(1)
                24: [("qk", 2, 2)], 25: [("qk", 6, 2)],
                26: [("qk", 3, 2)], 27: [("qk", 7, 2)],
                28: [("op", 4, 0)], 29: [("op", 4, 1)],
                30: [("op", 5, 0)], 31: [("op", 5, 1)],
                32: [("qk", 0, 3)], 33: [("qk", 4, 3)],
                34: [("v", 12)], 35: [("v", 13)],
                36: [("v", 14)], 37: [("v", 15)],
                38: [("qk", 1, 3)], 39: [("qk", 5, 3)],
                40: [("op", 6, 0)], 41: [("op", 6, 1)],
                42: [("qk", 2, 3)], 43: [("qk", 6, 3)],
                44: [("qk", 3, 3)], 45: [("qk", 7, 3)],
                46: [("op", 7, 0)], 47: [("op", 7, 1)],
                # J=3 (pops 48..79): oproj(1 rest) + oproj(2)
                48: [("op", 8, 0)], 50: [("op", 8, 1)],
                52: [("op", 9, 0)], 54: [("op", 9, 1)],
                56: [("op", 10, 0)], 58: [("op", 10, 1)],
                60: [("op", 11, 0)], 62: [("op", 11, 1)],
            }
            giter = [0]

            def pop_fill():
                g = giter[0]
                giter[0] += 1
                for u in sched.get(g, []):
                    if u[0] == "v":
                        emit_v(u[1])
                    elif u[0] == "qk":
                        emit_qk(u[1], u[2])
                    else:
                        emit_oproj(u[1], u[2])

            # ---- attention; O.T accumulated with V' stationary ----
            # two heads (one even, one odd parity) are software-pipelined:
            # while ScalarE exps head A's scores, PE runs head B's matmuls.
            def st_exp2(h0, h1, J, pair):
                ps0 = psS.tile([128, 1024], f32, tag="s",
                               name=f"ps{h0}_{J}_{pair[0]}")
                ps1 = psS.tile([128, 1024], f32, tag="s",
                               name=f"ps{h1}_{J}_{pair[0]}")
                pt0 = ptp.tile([128, 1024], bf16, tag="p",
                               name=f"pt{h0}_{J}_{pair[0]}")
                pt1 = ptp.tile([128, 1024], bf16, tag="p",
                               name=f"pt{h1}_{J}_{pair[0]}")
                QT0 = QKT[0:64, h0 // 2, :]
                KT0 = QKT[0:64, 4 + h0 // 2, :]
                QT1 = QKT[64:128, h1 // 2, :]
                KT1 = QKT[64:128, 4 + h1 // 2, :]
                col = 0
                offs = []
                for i in pair:
                    qlo = max(J * 512, i * 128)
                    span = (J + 1) * 512 - qlo
                    # each matmul region must stay within one bank
                    assert col // 512 == (col + span - 1) // 512
                    nc.tensor.matmul(
                        ps0[:, col:col + span],
                        lhsT=KT0[:, i * 128:(i + 1) * 128],
                        rhs=QT0[:, qlo:qlo + span],
                        start=True, stop=True,
                    )
                    nc.tensor.matmul(
                        ps1[:, col:col + span],
                        lhsT=KT1[:, i * 128:(i + 1) * 128],
                        rhs=QT1[:, qlo:qlo + span],
                        start=True, stop=True,
                    )
                    offs.append((i, col, qlo, span))
                    col += span
                # QKT holds 64*(Q|K): scores are 4096x -> fold into scale
                nc.scalar.activation(
                    out=pt0[:, 0:col], in_=ps0[:, 0:col], func=Exp,
                    scale=0.125 / (WSCALE * WSCALE))
                nc.scalar.activation(
                    out=pt1[:, 0:col], in_=ps1[:, 0:col], func=Exp,
                    scale=0.125 / (WSCALE * WSCALE))
                return (pt0, offs), (pt1, offs)

            def av(h, J, pt, offs, otr):
                for i, coff, qlo, span in offs:
                    if i >= 4 * J:  # diagonal: zero upper triangle
                        nc.vector.tensor_tensor(
                            out=pt[:, coff:coff + 128],
                            in0=pt[:, coff:coff + 128], in1=TRI[:],
                            op=mybir.AluOpType.mult,
                        )
                    # O.T[:, qloc:512] += V'_i.T @ P.T_i
                    qloc = qlo - J * 512
                    nc.tensor.matmul(
                        otr[:, qloc:512],
                        lhsT=VP[:, i, h, :],
                        rhs=pt[:, coff:coff + span],
                        start=(i == 0), stop=(i == 4 * J + 3),
                    )

            def normalize(h, J, otr):
                # O.T rows (base prow) times 1/den rows (base drow; all 64
                # denominator rows are identical by construction)
                prow = (h % 2) * 64
                drow = 64 - prow
                rd = dnp.tile([128, 512], f32, tag="d", name=f"rd{h}_{J}")
                if drow == 0:
                    nc.vector.reciprocal_approx_fast(
                        rd[0:64, :], otr[0:64, :])
                else:
                    # reciprocal_approx_fast needs its source at base 0
                    rdc = dnp.tile([128, 512], f32, tag="dc",
                                   name=f"rdc{h}_{J}")
                    nc.vector.tensor_copy(
                        rdc[0:64, :], otr[drow:drow + 64, :])
                    nc.vector.reciprocal_approx_fast(
                        rd[0:64, :], rdc[0:64, :])
                nc.vector.tensor_tensor(
                    out=OT[prow:prow + 64, h // 2, J * 512:(J + 1) * 512],
                    in0=otr[prow:prow + 64, :],
                    in1=rd[0:64, :],
                    op=mybir.AluOpType.mult,
                )

            oproj_q = []
            for J in range(T // 512):
                for hp in range(HPC // 2):
                    h0, h1 = 2 * hp, 2 * hp + 1
                    otr0 = psO.tile([128, 512], f32, tag="o",
                                    name=f"otr{h0}_{J}")
                    otr1 = psO.tile([128, 512], f32, tag="o",
                                    name=f"otr{h1}_{J}")
                    ks = list(range(4 * J + 4))
                    pairs = [ks[m:m + 2] for m in range(0, len(ks), 2)]
                    prev = None
                    for pair in pairs:
                        (pt0, offs0), (pt1, offs1) = st_exp2(h0, h1, J, pair)
                        pop_fill()
                        if prev is not None:
                            av(h0, J, prev[0][0], prev[0][1], otr0)
                            av(h1, J, prev[1][0], prev[1][1], otr1)
                        prev = ((pt0, offs0), (pt1, offs1))
                    av(h0, J, prev[0][0], prev[0][1], otr0)
                    # h0's normalize (DVE chain) overlaps h1's PV matmuls
                    normalize(h0, J, otr0)
                    av(h1, J, prev[1][0], prev[1][1], otr1)
                    normalize(h1, J, otr1)
                # chunk J is complete for all heads: queue its O-projection
                for tq in range(4 * J, 4 * J + 4):
                    for oc2 in range(D // 512):
                        oproj_q.append((tq, oc2))

            while oproj_q:
                emit_oproj(*oproj_q.pop(0))

    nc.compile()
    return nc


def _in_maps(x, W_qkv, b_qkv, W_o, b_o):
    x = np.asarray(x, np.float32)
    W_qkv = np.asarray(W_qkv, np.float32)
    b_qkv = np.asarray(b_qkv, np.float32)
    W_o = np.asarray(W_o, np.float32)
    b_o = np.asarray(b_o, np.float32)

    maps = []
    for c in range(N_CORES):
        b, hh = c // 2, c % 2
        rs = slice(hh * OC, (hh + 1) * OC)
        wq = W_qkv[0 * D:1 * D][rs]            # [512, 1024]
        wk = W_qkv[1 * D:2 * D][rs]
        wvv = W_qkv[2 * D:3 * D][rs]
        wqkT = np.concatenate([wq, wk], 0).T   # [1024, 1024]
        bq = b_qkv[0 * D:1 * D][rs]
        bk = b_qkv[1 * D:2 * D][rs]
        bvv = b_qkv[2 * D:3 * D][rs]
        tri = np.triu(np.ones((128, 128), np.float32))
        xTc = np.ascontiguousarray(x[b].T)
        maps.append({
            "xT": xTc.astype(BF16),
            "x8": xTc.astype(F8E4),
            "w8": np.ascontiguousarray(WSCALE * wqkT).astype(F8E4),
            "wvT": np.ascontiguousarray(wvv.T).astype(BF16),
            "woT": np.ascontiguousarray(W_o[:, rs].T).astype(BF16),
            "bqk": np.ascontiguousarray(
                (WSCALE * np.concatenate([bq, bk]))
                .reshape(2 * OC // 128, 128).T),
            "bv": bvv.reshape(1, OC),
            "bo": (0.5 * b_o).reshape(1, D),
            "tri": tri.astype(BF16),
        })
    return maps


def _run(x, W_qkv, b_qkv, W_o, b_o, trace=False, tmpdir=None):
    from concourse.bass_utils import run_bass_kernel_spmd

    if "nc" not in _cache:
        _cache["nc"] = _build()
    res = run_bass_kernel_spmd(
        _cache["nc"], _in_maps(x, W_qkv, b_qkv, W_o, b_o),
        core_ids=list(range(N_CORES)), trace=trace, tmpdir=tmpdir,
    )
    out = np.empty((B, T, D), np.float32)
    for b in range(B):
        out[b] = res.results[2 * b]["out"] + res.results[2 * b + 1]["out"]
    return out, res


def kernel(x, W_qkv, b_qkv, W_o, b_o):
    out, _ = _run(x, W_qkv, b_qkv, W_o, b_o, trace=False)
    return out
